# revision 41
# baseline (speedup 1.0000x reference)
"""Trainium2 Bass kernel for nn_EstVAEStudent (moe_routing).

Data-parallel over batch: 8 cores x 512 tokens, weights replicated.

v2: fp8e4 + DoubleRow matmuls (2x PE throughput, K=256/instruction) for the
conv encoder, hlin1, the 16 dense experts and the shared experts. The VAE
encoder, proj, router and output MLP stay float32r/fp32 (fp8 there flips
router selections). Expert down-projections run token-major so the routing
weights apply as per-partition scalars during PSUM eviction (no broadcast
DMAs), accumulating h2 = sum_e w_e y_e + w.bd + shared + sh_d_b in SBUF.
Router math is fp32 from an exact xcat copy with proj@router collapsed on
host; its top-k runs batched over all 512 tokens.

fp8 scaling: SX=8 (x), SW=32 (weights), so u-PSUMs sit at 256x and the
expert h stores as fp8 at 256*h (<240). Conv activations store at scale 1
(values <0.2), conv3 output + pooling in bf16.
"""

import os
import sys

sys.path.insert(0, "/opt/trn_rl_repo")

import numpy as np
import ml_dtypes

import concourse.bass as bass
import concourse.tile as tile
from concourse import bacc, mybir
from concourse.bass import ts
from concourse.bass_utils import run_bass_kernel_spmd
from concourse.masks import make_identity

F32 = mybir.dt.float32
F32R = mybir.dt.float32r
BF16 = mybir.dt.bfloat16
F8 = mybir.dt.float8e4
E4 = ml_dtypes.float8_e4m3
AF = mybir.ActivationFunctionType
ALU = mybir.AluOpType
AX = mybir.AxisListType
DR = mybir.MatmulPerfMode.DoubleRow

P = 128
NCORES = 8
B = 4096
BC = B // NCORES          # tokens per core = 512
TB = 128                  # conv token block
CUR_OBS = 256
HIST_C = 96
HIST_T = 25
FUT = 2560
HID = 1024
PROJ = 512
HLAT = 64
FLAT = 64
CONV1, CONV2, CONV3 = 256, 512, 1024
E = 16
NSH = 2
OUT = 23

SX = 8.0                  # x -> fp8 scale
SW = 32.0                 # fp8 weight scale
SXW = SX * SW             # u/g psum scale (256)
SYD = SXW * SW            # down-proj psum scale (8192)
SCIN = 16.0               # conv input scale
SPOOL = 256.0             # pooled -> fp8 scale

# bias-blob column layout: name -> (col, width)
_COFF = {}
_CB_COLS = 0
for _nm, _w in [("b1", 2), ("b2", 4), ("b3", 8), ("hl1b", 8), ("e1b", 8),
                ("e2b", 8), ("projb8", 4), ("bpr", E), ("bgb", E * 8),
                ("bub", E * 8), ("shgb", 16), ("shub", 16), ("o1b", 8),
                ("o2b", 8), ("hl2b", 1), ("e3mub", 1), ("e3lvb", 1),
                ("headb", 1)]:
    _COFF[_nm] = (_CB_COLS, _w)
    _CB_COLS += _w

_CACHE = {}


def _build(debug=False):
    nc = bacc.Bacc()
    dp = nc.declare_dram_parameter

    # ---- per-core inputs ----
    # host im2col conv1 input, block-contiguous: [block, ci, k, t'*TB+tok]
    xh1 = dp("xh1", (BC // TB, P, 3, HIST_T * TB), F8, isOutput=False)
    futt = dp("futt", (FUT, BC), F32, isOutput=False)
    obst = dp("obst", (CUR_OBS, BC), F32, isOutput=False)
    noiset = dp("noiset", (FLAT, BC), F32, isOutput=False)

    # ---- weights ----
    w18 = dp("w18", (P, 3, CONV1), F8, isOutput=False)    # [ci(96p), k, co]*SW
    w28 = dp("w28", (P, 3, 2, CONV2), F8, isOutput=False)
    w38 = dp("w38", (P, 3, 4, CONV3), F8, isOutput=False)
    hl1w8 = dp("hl1w8", (P, 8, HID), F8, isOutput=False)  # hlin1_w*SW (no /7)
    hl2w = dp("hl2w", (HID, HLAT), F32, isOutput=False)
    e1w = dp("e1w", (FUT, HID), F32, isOutput=False)
    e2w = dp("e2w", (HID, HID), F32, isOutput=False)
    e3w = dp("e3w", (HID, 2 * FLAT), F32, isOutput=False)
    projw = dp("projw", (CUR_OBS + HLAT + FLAT, PROJ), F32, isOutput=False)
    wpr = dp("wpr", (CUR_OBS + HLAT + FLAT, E), F32, isOutput=False)
    Wgu8 = dp("Wgu8", (E, P, 8, HID), F8, isOutput=False)  # [0:4]=Wg, [4:8]=Wu
    Wd8 = dp("Wd8", (E, P, 8, PROJ), F8, isOutput=False)
    shg8 = dp("shg8", (P, 4, NSH * HID), F8, isOutput=False)
    shu8 = dp("shu8", (P, 4, NSH * HID), F8, isOutput=False)
    shd8 = dp("shd8", (P, 16, PROJ), F8, isOutput=False)
    o1w = dp("o1w", (PROJ, HID), F32, isOutput=False)
    o2w = dp("o2w", (HID, HID), F32, isOutput=False)
    headw = dp("headw", (HID, OUT), F32, isOutput=False)

    # biases / consts packed into one blob (single DMA; see _COFF)
    cblob = dp("cblob", (P, _CB_COLS), F32, isOutput=False)
    bde17 = dp("bde17", (17, PROJ), F32, isOutput=False)  # [bd; sh_d_b]*SYD^2
    onesrow = dp("onesrow", (1, BC), F32, isOutput=False)  # 1/SYD

    outp = dp("outp", (OUT, BC), F32, isOutput=True)
    dbg = {}
    if debug:
        dbg["w"] = dp("dbg_w", (P, 4, E), F32, isOutput=True)
        dbg["h2"] = dp("dbg_h2", (P, 4, BC), F32, isOutput=True)
        dbg["x8"] = dp("dbg_x8", (P, 4, BC), F32, isOutput=True)
        dbg["macc"] = dp("dbg_macc", (P, 4, PROJ), F32, isOutput=True)

    with tile.TileContext(nc, pool_alloc_mode="queue") as tc:
        _emit(nc, tc, locals(), dbg)
    nc.compile()
    return nc


def _emit(nc, tc, t, dbg):
    import contextlib

    ctx = contextlib.ExitStack()
    with ctx:
        const = ctx.enter_context(tc.tile_pool(name="const", bufs=1))
        wpool = ctx.enter_context(tc.tile_pool(name="wpool", bufs=3))
        e1pool = ctx.enter_context(tc.tile_pool(name="e1pool", bufs=2))
        acts = ctx.enter_context(tc.tile_pool(name="acts", bufs=1))
        ps = ctx.enter_context(tc.tile_pool(name="ps", bufs=4, space="PSUM"))

        # ---------- constants (single blob DMA) ----------
        cb = const.tile([P, _CB_COLS], F32, tag="cblob")
        nc.sync.dma_start(cb[:], t["cblob"][:])

        def cbv(name, rows=P):
            a, w = _COFF[name]
            return cb[0:rows, a:a + w]

        b1s = cbv("b1"); b2s = cbv("b2"); b3s = cbv("b3")
        hl1bs = cbv("hl1b"); hl2bs = cbv("hl2b", HLAT)
        e1bs = cbv("e1b"); e2bs = cbv("e2b")
        e3mubs = cbv("e3mub", FLAT); e3lvbs = cbv("e3lvb", FLAT)
        projb8s = cbv("projb8"); bprs = cbv("bpr")
        bgbs = cbv("bgb").rearrange("p (e i) -> p e i", i=8)
        bubs = cbv("bub").rearrange("p (e i) -> p e i", i=8)
        shgbs = cbv("shgb"); shubs = cbv("shub")
        o1bs = cbv("o1b"); o2bs = cbv("o2b")
        headbs = cbv("headb", OUT)
        ident = const.tile([P, P], F32, tag="ident")
        make_identity(nc, ident)

        # bd-init rhs (f32r for 1cy/row); DMAs deferred past the conv weights
        bde17s = const.tile([17, PROJ], F32R, tag="bde17")
        # wTs17: transposed routing weights (rows 0-15) + const row 16 = 1/SYD
        wTs17 = acts.tile([17, BC], F32R, tag="wTs17")

        # persistent activations
        xcat = acts.tile([P, 3, BC], F32R, tag="xcat")
        xcat32 = acts.tile([P, 3, BC], F32, tag="xcat32")   # exact fp32 for router
        x8 = acts.tile([P, 4, BC], F8, tag="x8")

        # ---------- conv encoder (fp8 DR) + VAE e1 (f32r) interleaved ----------
        histp_cm = tc.tile_pool(name="histp", bufs=1)
        histp = histp_cm.__enter__()
        pooled = histp.tile([P, 8, BC], BF16, tag="pooled")
        vaep_cm = tc.tile_pool(name="vaep", bufs=1)
        vaep = vaep_cm.__enter__()
        futs = vaep.tile([P, FUT // P, BC], F32R, tag="futs")
        ee1 = vaep.tile([P, 8, BC], F32R, tag="ee1")

        e2pre = {}

        def emit_e1_mtile(m):
            nko = FUT // P
            wt = e1pool.tile([P, nko, P], F32R, tag="w", name="e1wt")
            nc.sync.dma_start(
                wt[:], t["e1w"].rearrange("(o p) m -> p o m", p=P)
                [:, :, ts(m, P)].bitcast(F32R))
            acc = ps.tile([P, 512], F32, tag="ps", name="acc")
            for ko in range(nko):
                nc.tensor.matmul(acc[:], lhsT=wt[:, ko, :],
                                 rhs=futs[:, ko, :],
                                 start=(ko == 0), stop=(ko == nko - 1))
            nc.scalar.activation(ee1[:, m, :], acc[:], AF.Silu, bias=e1bs[:, m:m + 1])

        with tc.tile_pool(name="convp", bufs=1) as convp, \
             tc.tile_pool(name="cps", bufs=2, space="PSUM") as cps:
            w1s = convp.tile([P, 3, CONV1], F8, tag="w1s")
            nc.sync.dma_start(w1s[:], t["w18"][:])
            w2s = convp.tile([P, 3, 2, CONV2], F8, tag="w2s")
            nc.sync.dma_start(w2s[:], t["w28"][:])
            w3s = convp.tile([P, 3, 4, CONV3], F8, tag="w3s")
            nc.sync.dma_start(w3s[:], t["w38"][:])

            for b in range(BC // TB):
                # host-im2col conv1 input: [ci(96), k, c = t'*TB + tok]
                x1 = convp.tile([P, 3, 25 * TB], F8, tag="cX")
                nc.sync.dma_start(x1[:], t["xh1"][b])
                if b == 0:
                    for fc in range(4):
                        nc.sync.dma_start(
                            futs[:, ts(fc, 5), :],
                            t["futt"].rearrange("(o p) n -> p o n", p=P)[:, ts(fc, 5), :].bitcast(F32R))
                    nc.sync.dma_start(
                        xcat[:, 0:2, :],
                        t["obst"].rearrange("(o p) n -> p o n", p=P).bitcast(F32R))
                    nc.sync.dma_start(
                        xcat32[:, 0:2, :],
                        t["obst"].rearrange("(o p) n -> p o n", p=P))
                    nc.sync.dma_start(bde17s[:], t["bde17"][:].bitcast(F32R))
                    nc.sync.dma_start(wTs17[16:17, :], t["onesrow"][:].bitcast(F32R))

                # conv1 out, t-major with pad rows 0 and 26: [co-tile, (t'+1)*TB + tok]
                s1t = convp.tile([P, 2, 27 * TB], F8, tag="cA")
                nc.vector.memset(s1t[:, :, 0:TB], 0.0)
                nc.vector.memset(s1t[:, :, 26 * TB:27 * TB], 0.0)
                C1 = 25 * TB
                for co in range(2):
                    for cbase in range(0, C1, 1024):
                        w = min(1024, C1 - cbase)
                        acc = cps.tile([P, 1024], F32, tag="cps", name="acc")
                        for c0 in range(0, w, 512):
                            cw = min(512, w - c0)
                            sub = acc[:, c0:c0 + cw]
                            nc.tensor.matmul(sub, lhsT=w1s[0:96, 0:2, ts(co, P)],
                                             rhs=x1[0:96, 0:2, cbase + c0:cbase + c0 + cw],
                                             start=True, stop=False, perf_mode=DR,
                                             skip_group_check=True)
                            nc.tensor.matmul(sub, lhsT=w1s[0:96, 2, ts(co, P)],
                                             rhs=x1[0:96, 2, cbase + c0:cbase + c0 + cw],
                                             start=False, stop=True, skip_group_check=True)
                        nc.scalar.activation(s1t[:, co, TB + cbase:TB + cbase + w],
                                             acc[:, 0:w], AF.Silu,
                                             bias=b1s[:, co:co + 1], scale=1.0 / (SCIN * SW))

                # on-chip im2col for conv2: s1im[ci, k, t''*TB+tok] = s1t[ci, (2t''+k)*TB+tok]
                # one 4D-AP DMA per ci (k, t'', tok), issued from the idle gpsimd DGE
                s1im = convp.tile([P, 2, 3, 13 * TB], F8, tag="cX")
                for ci in range(2):
                    src = bass.AP(tensor=s1t.tensor,
                                  offset=s1t.offset + ci * (27 * TB),
                                  ap=[[1, P], [TB, 3], [2 * TB, 13], [1, TB]])
                    nc.gpsimd.dma_start(
                        s1im[:, ci].rearrange("p k (t n) -> p k t n", n=TB), src)

                # conv2: stride 2, ci 256 = DR pair over partition tiles
                s2t = convp.tile([P, 4, 15 * TB], F8, tag="cB")
                nc.vector.memset(s2t[:, :, 0:TB], 0.0)
                nc.vector.memset(s2t[:, :, 14 * TB:15 * TB], 0.0)
                C2 = 13 * TB
                for co in range(4):
                    for cbase in range(0, C2, 1024):
                        w = min(1024, C2 - cbase)
                        acc = cps.tile([P, 1024], F32, tag="cps", name="acc")
                        for c0 in range(0, w, 512):
                            cw = min(512, w - c0)
                            for k in range(3):
                                nc.tensor.matmul(
                                    acc[:, c0:c0 + cw], lhsT=w2s[:, k, :, ts(co, P)],
                                    rhs=s1im[:, :, k, cbase + c0:cbase + c0 + cw],
                                    start=(k == 0), stop=(k == 2), perf_mode=DR,
                                    skip_group_check=True)
                        nc.scalar.activation(s2t[:, co, TB + cbase:TB + cbase + w],
                                             acc[:, 0:w], AF.Silu,
                                             bias=b2s[:, co:co + 1], scale=1.0 / SW)

                # im2col for conv3
                s2im = convp.tile([P, 4, 3, 7 * TB], F8, tag="cA")
                for ci in range(4):
                    src = bass.AP(tensor=s2t.tensor,
                                  offset=s2t.offset + ci * (15 * TB),
                                  ap=[[1, P], [TB, 3], [2 * TB, 7], [1, TB]])
                    nc.gpsimd.dma_start(
                        s2im[:, ci].rearrange("p k (t n) -> p k t n", n=TB), src)

                # conv3: stride 2, ci 512 = 2 DR pairs
                s3 = convp.tile([P, 8, 7 * TB], BF16, tag="cB")
                C3 = 7 * TB
                for co in range(8):
                    acc = cps.tile([P, 1024], F32, tag="cps", name="acc")
                    for c0 in range(0, C3, 512):
                        cw = min(512, C3 - c0)
                        first = True
                        for cp in range(2):
                            for k in range(3):
                                nc.tensor.matmul(
                                    acc[:, c0:c0 + cw],
                                    lhsT=w3s[:, k, slice(2 * cp, 2 * cp + 2), ts(co, P)],
                                    rhs=s2im[:, slice(2 * cp, 2 * cp + 2), k, c0:c0 + cw],
                                    start=first, stop=(cp == 1 and k == 2), perf_mode=DR,
                                    skip_group_check=True)
                                first = False
                    nc.scalar.activation(s3[:, co, :], acc[:, 0:C3], AF.Silu,
                                         bias=b3s[:, co:co + 1], scale=1.0 / SW)
                # pool (sum over t; /7 folded into pooled8 cast)
                s3v = s3.rearrange("p c (t n) -> p c t n", n=TB)
                pb = pooled[:, :, ts(b, TB)]
                with nc.allow_low_precision(reason="bf16 pooling"):
                    nc.vector.tensor_add(pb, s3v[:, :, 0, :], s3v[:, :, 1, :])
                    for tt_ in range(2, 7):
                        nc.vector.tensor_add(pb, pb, s3v[:, :, tt_, :])
                emit_e1_mtile(2 * b)
                emit_e1_mtile(2 * b + 1)
                if b == 2:
                    for m_ in range(3):
                        wt = wpool.tile([P, 8, P], F32R, tag="wsm", name="wt")
                        nc.sync.dma_start(
                            wt[:], t["e2w"].rearrange("(o p) m -> p o m", p=P)
                            [:, :, ts(m_, P)].bitcast(F32R))
                        e2pre[m_] = wt

        # post-conv persistent pool (reuses the conv pools' SBUF space);
        # right-side stack so it can outlive the left-side histp/vaep pools
        acts2 = ctx.enter_context(tc.tile_pool(name="acts2", bufs=1, side="right"))
        hl1w8s = acts2.tile([P, 8, HID], F8, tag="hl1w8")
        nc.sync.dma_start(hl1w8s[:], t["hl1w8"][:])
        shg8s = acts2.tile([P, 4, NSH * HID], F8, tag="shg8")
        nc.sync.dma_start(shg8s[:], t["shg8"][:])
        shu8s = acts2.tile([P, 4, NSH * HID], F8, tag="shu8")
        nc.sync.dma_start(shu8s[:], t["shu8"][:])
        shd8s = acts2.tile([P, 16, PROJ], F8, tag="shd8")
        nc.sync.dma_start(shd8s[:], t["shd8"][:])

        # ---------- VAE e2/e3 (f32r) + hist MLP ----------
        with tc.tile_pool(name="vtmp", bufs=1) as vtmp:
            ee2 = vtmp.tile([P, 8, BC], F32R, tag="ee2")
            for m in range(8):
                if m in e2pre:
                    wt = e2pre[m]
                else:
                    wt = wpool.tile([P, 8, P], F32R, tag="wsm", name="wt")
                    nc.sync.dma_start(
                        wt[:], t["e2w"].rearrange("(o p) m -> p o m", p=P)[:, :, ts(m, P)].bitcast(F32R))
                acc = ps.tile([P, 512], F32, tag="ps", name="acc")
                for ko in range(8):
                    nc.tensor.matmul(acc[:], lhsT=wt[:, ko, :], rhs=ee1[:, ko, :],
                                     start=(ko == 0), stop=(ko == 7))
                nc.scalar.activation(ee2[:, m, :], acc[:], AF.Silu, bias=e2bs[:, m:m + 1])
            # hist MLP: pooled -> fp8 cast, hl1 in fp8-DR, hl2 f32r
            pooled8 = vtmp.tile([P, 8, BC], F8, tag="pooled8")
            nc.vector.tensor_scalar_mul(pooled8[:], pooled[:], SPOOL / 7.0)
            h1 = vtmp.tile([P, 8, BC], F32R, tag="h1")
            for m in range(8):
                acc = ps.tile([P, 512], F32, tag="ps", name="acc")
                for j in range(4):
                    nc.tensor.matmul(acc[:], lhsT=hl1w8s[:, 2 * j:2 * j + 2, ts(m, P)],
                                     rhs=pooled8[:, 2 * j:2 * j + 2, :],
                                     start=(j == 0), stop=(j == 3), perf_mode=DR)
                nc.scalar.activation(h1[:, m, :], acc[:], AF.Silu, bias=hl1bs[:, m:m + 1],
                                     scale=1.0 / (SPOOL * SW))
            wt = wpool.tile([P, 8, HLAT], F32R, tag="wsm", name="wt")
            nc.sync.dma_start(wt[:], t["hl2w"].rearrange("(o p) m -> p o m", p=P).bitcast(F32R))
            acc = ps.tile([P, 512], F32, tag="ps", name="acc")[:HLAT, :]
            for ko in range(8):
                nc.tensor.matmul(acc, lhsT=wt[:, ko, :], rhs=h1[:, ko, :],
                                 start=(ko == 0), stop=(ko == 7))
            nc.scalar.activation(xcat32[0:HLAT, 2, :], acc, AF.Identity, bias=hl2bs[:])
            nc.vector.tensor_copy(xcat[0:HLAT, 2, :], xcat32[0:HLAT, 2, :])
            # enc3 -> mu / logvar
            wt = wpool.tile([P, 8, 2 * FLAT], F32R, tag="wsm", name="wt")
            nc.sync.dma_start(wt[:], t["e3w"].rearrange("(o p) m -> p o m", p=P).bitcast(F32R))
            accmu = ps.tile([P, 512], F32, tag="ps", name="acc")[:FLAT, :]
            for ko in range(8):
                nc.tensor.matmul(accmu, lhsT=wt[:, ko, 0:FLAT], rhs=ee2[:, ko, :],
                                 start=(ko == 0), stop=(ko == 7))
            mus = vtmp.tile([FLAT, BC], F32, tag="mu")
            nc.scalar.activation(mus[:], accmu, AF.Identity, bias=e3mubs[:])
            acclv = ps.tile([P, 512], F32, tag="ps", name="acc")[:FLAT, :]
            for ko in range(8):
                nc.tensor.matmul(acclv, lhsT=wt[:, ko, FLAT:2 * FLAT], rhs=ee2[:, ko, :],
                                 start=(ko == 0), stop=(ko == 7))
            zexp = vtmp.tile([FLAT, BC], F32, tag="zexp")
            nc.scalar.activation(zexp[:], acclv, AF.Exp, bias=e3lvbs[:], scale=0.5)
            noises = vtmp.tile([FLAT, BC], F32, tag="noise")
            nc.sync.dma_start(noises[:], t["noiset"][:])
            nc.vector.tensor_mul(zexp[:], zexp[:], noises[:])
            nc.vector.tensor_add(zexp[:], zexp[:], mus[:])
            nc.sync.dma_start(xcat[HLAT:P, 2, :], zexp[:].bitcast(F32R))
            nc.sync.dma_start(xcat32[HLAT:P, 2, :], zexp[:])
        vaep_cm.__exit__(None, None, None)
        histp_cm.__exit__(None, None, None)

        # ---------- projection -> x8 ----------
        for m in range(4):
            wt = wpool.tile([P, 3, P], F32R, tag="wsm", name="wt")
            nc.sync.dma_start(
                wt[:], t["projw"].rearrange("(o p) m -> p o m", p=P)[:, :, ts(m, P)].bitcast(F32R))
            acc = ps.tile([P, 512], F32, tag="ps")
            for ko in range(3):
                nc.tensor.matmul(acc[:], lhsT=wt[:, ko, :], rhs=xcat[:, ko, :],
                                 start=(ko == 0), stop=(ko == 2))
            nc.scalar.activation(x8[:, m, :], acc[:], AF.Identity,
                                 bias=projb8s[:, m:m + 1], scale=SX)

        if dbg:
            x8dbg = acts2.tile([P, 4, BC], F32, tag="dbgx8", name="x8dbg")
            nc.scalar.activation(x8dbg[:], x8[:], AF.Identity, scale=1.0 / SX)
            nc.sync.dma_start(dbg["x8"][:], x8dbg[:])

        # ---------- router (fp32, batched over the 4 token tiles) ----------
        rws = const.tile([P, 3, E], F32, tag="rws")
        nc.sync.dma_start(rws[:], t["wpr"].rearrange("(o p) m -> p o m", p=P))
        wfin = acts.tile([P, 4, E], F32, tag="wfin")
        with tc.tile_pool(name="routp", bufs=1) as routp, \
             tc.tile_pool(name="pss", bufs=2, space="PSUM") as pss:
            sc = routp.tile([P, 4, E], F32, tag="sc")
            for tt in range(4):
                acc = pss.tile([P, E], F32, tag="rps")
                for ko in range(3):
                    nc.tensor.matmul(acc[:], lhsT=xcat32[:, ko, ts(tt, P)], rhs=rws[:, ko, :],
                                     start=(ko == 0), stop=(ko == 2))
                nc.vector.tensor_add(sc[:, tt, :], acc[:], bprs[:])
            nc.scalar.activation(sc[:], sc[:], AF.Sigmoid)
            scg = sc.rearrange("p f (g i) -> p f g i", g=4)
            # group score: sum of top2 of 4 = max over pairwise sums
            pa = routp.tile([P, 4, 4, 3], F32, tag="pa")
            nc.vector.tensor_add(pa[:], scg[:, :, :, 0:3], scg[:, :, :, 1:4])
            pb = routp.tile([P, 4, 4, 2], F32, tag="pb")
            nc.vector.tensor_add(pb[:], scg[:, :, :, 0:2], scg[:, :, :, 2:4])
            pc = routp.tile([P, 4, 4, 1], F32, tag="pc")
            nc.vector.tensor_add(pc[:], scg[:, :, :, 0:1], scg[:, :, :, 3:4])
            gsc = routp.tile([P, 4, 4], F32, tag="gsc")
            nc.vector.reduce_max(gsc[:], pa[:], axis=AX.X)
            pbm = routp.tile([P, 4, 4], F32, tag="pbm")
            nc.vector.reduce_max(pbm[:], pb[:], axis=AX.X)
            nc.vector.tensor_max(gsc[:], gsc[:], pbm[:])
            nc.vector.tensor_max(gsc[:], gsc[:], pc[:, :, :, 0])
            # top-2 groups by count-greater
            cg = routp.tile([P, 4, 4], F32, tag="cg")
            nc.vector.memset(cg[:], 0.0)
            tmp = routp.tile([P, 4, 4], F32, tag="tmpr")
            for d in range(1, 4):
                nc.vector.tensor_tensor(tmp[:, :, :4 - d], gsc[:, :, d:], gsc[:, :, :4 - d], ALU.is_gt)
                nc.vector.tensor_add(cg[:, :, :4 - d], cg[:, :, :4 - d], tmp[:, :, :4 - d])
                nc.vector.tensor_tensor(tmp[:, :, :4 - d], gsc[:, :, :4 - d], gsc[:, :, d:], ALU.is_gt)
                nc.vector.tensor_add(cg[:, :, d:], cg[:, :, d:], tmp[:, :, :4 - d])
            gmask = routp.tile([P, 4, 4], F32, tag="gmask")
            nc.vector.tensor_scalar(gmask[:], cg[:], 1.5, None, ALU.is_le)
            msk = routp.tile([P, 4, 4, 4], F32, tag="msk")
            nc.vector.tensor_tensor(msk[:], scg[:], gmask[:, :, :, None].to_broadcast((P, 4, 4, 4)), ALU.mult)
            mskf = msk.rearrange("p f g i -> p f (g i)")
            # top-4 of 16 by count-greater
            cg16 = routp.tile([P, 4, E], F32, tag="cg16")
            nc.vector.memset(cg16[:], 0.0)
            t16 = routp.tile([P, 4, E], F32, tag="t16")
            for d in range(1, 16):
                nc.vector.tensor_tensor(t16[:, :, :E - d], mskf[:, :, d:], mskf[:, :, :E - d], ALU.is_gt)
                nc.vector.tensor_add(cg16[:, :, :E - d], cg16[:, :, :E - d], t16[:, :, :E - d])
                nc.vector.tensor_tensor(t16[:, :, :E - d], mskf[:, :, :E - d], mskf[:, :, d:], ALU.is_gt)
                nc.vector.tensor_add(cg16[:, :, d:], cg16[:, :, d:], t16[:, :, :E - d])
            sel = routp.tile([P, 4, E], F32, tag="sel")
            nc.vector.tensor_scalar(sel[:], cg16[:], 3.5, None, ALU.is_le)
            wsel = routp.tile([P, 4, E], F32, tag="wsel")
            nc.vector.tensor_tensor(wsel[:], mskf[:], sel[:], ALU.mult)
            den = routp.tile([P, 4], F32, tag="den")
            nc.vector.reduce_sum(den[:], wsel[:], axis=AX.X)
            nc.vector.tensor_scalar_add(den[:], den[:], 1e-20)
            nc.vector.tensor_scalar_mul(den[:], den[:], SYD)
            rec = routp.tile([P, 4], F32, tag="rec")
            nc.vector.reciprocal(rec[:], den[:])
            # wfin = wsel / (SYD * den)  (per-token scale broadcast over E)
            nc.vector.tensor_tensor(wfin[:], wsel[:], rec[:, :, None].to_broadcast((P, 4, E)), ALU.mult)
            # transpose routing weights into wTs17 rows 0-15
            for tt in range(4):
                acct = pss.tile([E, P], F32, tag="tps")
                nc.tensor.transpose(acct[:], wfin[:, tt, :], ident[:])
                with nc.allow_low_precision(reason="f32r copy of router weights"):
                    nc.vector.tensor_copy(wTs17[0:16, ts(tt, P)], acct[:].bitcast(F32R))

        if dbg:
            nc.sync.dma_start(dbg["w"][:], wfin[:])

        # ---------- shared experts (fp8 DR) + bd-init, token-major down ----------
        # psum pool for token-major down-projections (router pss closed by now)
        psy = ctx.enter_context(tc.tile_pool(name="psy", bufs=2, space="PSUM"))
        macc = acts2.tile([P, 4, PROJ], F32, tag="macc")  # [tok, tt, proj]
        with tc.tile_pool(name="hshp", bufs=1) as hshp, \
             tc.tile_pool(name="hgp", bufs=3) as hgp:
            hsh8 = hshp.tile([P, 16, BC], F8, tag="hsh8")
            for i in range(16):
                gacc = ps.tile([P, 512], F32, tag="ps")
                for j in range(2):
                    nc.tensor.matmul(gacc[:], lhsT=shg8s[:, 2 * j:2 * j + 2, ts(i, P)],
                                     rhs=x8[:, 2 * j:2 * j + 2, :],
                                     start=(j == 0), stop=(j == 1), perf_mode=DR)
                hgt = hgp.tile([P, BC], F32, tag="hgt")
                nc.scalar.activation(hgt[:], gacc[:], AF.Silu, bias=shgbs[:, i:i + 1],
                                     scale=1.0 / SXW)
                uacc = ps.tile([P, 512], F32, tag="ps")
                for j in range(2):
                    nc.tensor.matmul(uacc[:], lhsT=shu8s[:, 2 * j:2 * j + 2, ts(i, P)],
                                     rhs=x8[:, 2 * j:2 * j + 2, :],
                                     start=(j == 0), stop=(j == 1), perf_mode=DR)
                nc.vector.scalar_tensor_tensor(hsh8[:, i, :], uacc[:], shubs[:, i:i + 1],
                                               hgt[:], ALU.add, ALU.mult)
            # token-major: psum_tt = bd-init (K=17) + shared down; evict -> macc
            for tt in range(4):
                yacc = psy.tile([P, 512], F32, tag="psy")
                nc.tensor.matmul(yacc[:], lhsT=wTs17[:, ts(tt, P)], rhs=bde17s[:],
                                 start=True, stop=False, skip_group_check=True)
                for j in range(8):
                    nc.tensor.matmul(yacc[:], lhsT=hsh8[:, 2 * j:2 * j + 2, ts(tt, P)],
                                     rhs=shd8s[:, 2 * j:2 * j + 2, :],
                                     start=False, stop=(j == 7), perf_mode=DR,
                                     skip_group_check=True)
                nc.scalar.activation(macc[:, tt, :], yacc[:], AF.Identity, scale=1.0 / SYD)

        # ---------- 16 experts (fp8 DR), token-major down + fused routing ----------
        with tc.tile_pool(name="moep", bufs=3) as moep, \
             tc.tile_pool(name="h8p", bufs=2) as h8p, \
             tc.tile_pool(name="hgp2", bufs=3) as hgp2:
            for e in range(E):
                wgu = moep.tile([P, 8, HID], F8, tag="wgu")
                nc.sync.dma_start(wgu[:], t["Wgu8"][e])
                wg = wgu[:, 0:4, :]
                wu = wgu[:, 4:8, :]
                wd = moep.tile([P, 8, PROJ], F8, tag="wd")
                nc.sync.dma_start(wd[:], t["Wd8"][e])
                h8 = h8p.tile([P, 8, BC], F8, tag="h8")
                for i in range(8):
                    gacc = ps.tile([P, 512], F32, tag="ps")
                    for j in range(2):
                        nc.tensor.matmul(gacc[:], lhsT=wg[:, 2 * j:2 * j + 2, ts(i, P)],
                                         rhs=x8[:, 2 * j:2 * j + 2, :],
                                         start=(j == 0), stop=(j == 1), perf_mode=DR)
                    hgt = hgp2.tile([P, BC], F32, tag="hgt")
                    nc.scalar.activation(hgt[:], gacc[:], AF.Silu, bias=bgbs[:, e, i:i + 1],
                                         scale=1.0 / SXW)
                    uacc = ps.tile([P, 512], F32, tag="ps")
                    for j in range(2):
                        nc.tensor.matmul(uacc[:], lhsT=wu[:, 2 * j:2 * j + 2, ts(i, P)],
                                         rhs=x8[:, 2 * j:2 * j + 2, :],
                                         start=(j == 0), stop=(j == 1), perf_mode=DR)
                    nc.vector.scalar_tensor_tensor(h8[:, i, :], uacc[:], bubs[:, e, i:i + 1],
                                                   hgt[:], ALU.add, ALU.mult)
                for tt in range(4):
                    yacc = psy.tile([P, 512], F32, tag="psy")
                    for j in range(4):
                        nc.tensor.matmul(yacc[:], lhsT=h8[:, 2 * j:2 * j + 2, ts(tt, P)],
                                         rhs=wd[:, 2 * j:2 * j + 2, :],
                                         start=(j == 0), stop=(j == 3), perf_mode=DR)
                    # macc += y * w_e(token)   (w pre-divided by SYD)
                    nc.vector.scalar_tensor_tensor(macc[:, tt, :], yacc[:],
                                                   wfin[:, tt, e:e + 1], macc[:, tt, :],
                                                   ALU.mult, ALU.add)

        if dbg:
            nc.sync.dma_start(dbg["macc"][:], macc[:])

        # ---------- transpose h2 to feature-major ----------
        h2 = acts2.tile([P, 4, BC], F32R, tag="h2")
        with tc.tile_pool(name="pst", bufs=2, space="PSUM") as pstp:
            for tt in range(4):
                for fc in range(4):
                    pst = pstp.tile([P, P], F32, tag="pst")
                    nc.tensor.transpose(pst[:], macc[:, tt, ts(fc, P)], ident[:])
                    nc.scalar.activation(h2[:, fc, ts(tt, P)], pst[:], AF.Identity)

        if dbg:
            nc.sync.dma_start(dbg["h2"][:], h2[:].bitcast(F32))

        # ---------- output MLP + head (f32r) ----------
        with tc.tile_pool(name="outp_", bufs=2) as outp_:
            o1 = outp_.tile([P, 8, BC], F32R, tag="o")
            for m in range(8):
                wt = wpool.tile([P, 4, P], F32R, tag="wsm", name="wt")
                nc.sync.dma_start(
                    wt[:], t["o1w"].rearrange("(o p) m -> p o m", p=P)[:, :, ts(m, P)].bitcast(F32R))
                acc = ps.tile([P, 512], F32, tag="ps")
                for ko in range(4):
                    nc.tensor.matmul(acc[:], lhsT=wt[:, ko, :], rhs=h2[:, ko, :],
                                     start=(ko == 0), stop=(ko == 3))
                nc.scalar.activation(o1[:, m, :], acc[:], AF.Silu, bias=o1bs[:, m:m + 1])
            o2 = outp_.tile([P, 8, BC], F32R, tag="o")
            for m in range(8):
                wt = wpool.tile([P, 8, P], F32R, tag="wsm", name="wt")
                nc.sync.dma_start(
                    wt[:], t["o2w"].rearrange("(o p) m -> p o m", p=P)[:, :, ts(m, P)].bitcast(F32R))
                acc = ps.tile([P, 512], F32, tag="ps")
                for ko in range(8):
                    nc.tensor.matmul(acc[:], lhsT=wt[:, ko, :], rhs=o1[:, ko, :],
                                     start=(ko == 0), stop=(ko == 7))
                nc.scalar.activation(o2[:, m, :], acc[:], AF.Identity, bias=o2bs[:, m:m + 1])
            wt = wpool.tile([P, 8, OUT], F32R, tag="wsm", name="wt")
            nc.sync.dma_start(wt[:], t["headw"].rearrange("(o p) m -> p o m", p=P).bitcast(F32R))
            acc = ps.tile([P, 512], F32, tag="ps", name="acc")[:OUT, :]
            for ko in range(8):
                nc.tensor.matmul(acc, lhsT=wt[:, ko, :], rhs=o2[:, ko, :],
                                 start=(ko == 0), stop=(ko == 7))
            outs = outp_.tile([OUT, BC], F32, tag="outs")
            nc.scalar.activation(outs[:], acc, AF.Identity, bias=headbs[:])
            nc.sync.dma_start(t["outp"][:], outs[:])


def _colmajor(v, ntiles):
    return np.ascontiguousarray(v.reshape(ntiles, P).T)


def _q8(a, s):
    return np.asarray(np.asarray(a, np.float32) * s, E4)


def _prep(inputs):
    f = {k: np.ascontiguousarray(np.asarray(v, dtype=np.float32)) for k, v in inputs.items()}
    sh = {}
    # hist: [B,25,96] -> padded [128(ci), 27(t), B], then host im2col
    # xh1[ci, k, t', tok] = xpad[ci, t' + k, tok], quantized fp8 * SCIN
    xh = np.zeros((P, 27, B), np.float32)
    xh[:HIST_C, 1:26, :] = f["cur_hist_seq"].transpose(2, 1, 0)
    xim = np.stack([xh[:, k:k + HIST_T, :] for k in range(3)], axis=1)  # [P,3,25,B]
    # -> [core, block, P, 3, 25*TB] block-contiguous
    xim = xim.reshape(P, 3, HIST_T, NCORES, BC // TB, TB).transpose(3, 4, 0, 1, 2, 5)
    xh1 = _q8(np.ascontiguousarray(xim.reshape(NCORES, BC // TB, P, 3, HIST_T * TB)), SCIN)
    # conv weights: [co,ci,k] -> [ci(part), k, co] * SW
    w1 = np.zeros((P, 3, CONV1), np.float32)
    w1[:HIST_C] = f["conv1_w"].transpose(1, 2, 0)
    sh["w18"] = _q8(w1, SW)
    sh["w28"] = _q8(np.ascontiguousarray(
        f["conv2_w"].transpose(1, 2, 0).reshape(2, P, 3, CONV2).transpose(1, 2, 0, 3)), SW)
    sh["w38"] = _q8(np.ascontiguousarray(
        f["conv3_w"].transpose(1, 2, 0).reshape(4, P, 3, CONV3).transpose(1, 2, 0, 3)), SW)
    sh["hl1w8"] = _q8(np.ascontiguousarray(
        f["hlin1_w"].reshape(8, P, HID).transpose(1, 0, 2)), SW)
    sh["hl2w"] = f["hlin2_w"]
    sh["e1w"] = f["enc1_w"]; sh["e2w"] = f["enc2_w"]; sh["e3w"] = f["enc3_w"]
    sh["projw"] = f["proj_w"]
    sh["wpr"] = (f["proj_w"].astype(np.float64) @ f["router_w"].astype(np.float64)).astype(np.float32)
    wgu = np.concatenate([f["Wg"].reshape(E, 4, P, HID), f["Wu"].reshape(E, 4, P, HID)],
                         axis=1)
    sh["Wgu8"] = _q8(wgu.transpose(0, 2, 1, 3), SW)
    sh["Wd8"] = _q8(f["Wd"].reshape(E, 8, P, PROJ).transpose(0, 2, 1, 3), SW)
    sh["shg8"] = _q8(f["sh_g_w"].reshape(4, P, NSH * HID).transpose(1, 0, 2), SW)
    sh["shu8"] = _q8(f["sh_u_w"].reshape(4, P, NSH * HID).transpose(1, 0, 2), SW)
    sh["shd8"] = _q8(f["sh_d_w"].reshape(16, P, PROJ).transpose(1, 0, 2), SW)
    sh["o1w"] = f["out1_w"]; sh["o2w"] = f["out2_w"]; sh["headw"] = f["head_w"]
    cb = np.zeros((P, _CB_COLS), np.float32)

    def cput(name, arr, rows=P):
        a, w = _COFF[name]
        cb[0:rows, a:a + w] = arr.reshape(rows, w)

    cput("b1", _colmajor(f["conv1_b"], 2))
    cput("b2", _colmajor(f["conv2_b"], 4))
    cput("b3", _colmajor(f["conv3_b"], 8))
    cput("hl1b", _colmajor(f["hlin1_b"], 8))
    cput("hl2b", f["hlin2_b"], HLAT)
    cput("e1b", _colmajor(f["enc1_b"], 8))
    cput("e2b", _colmajor(f["enc2_b"], 8))
    cput("e3mub", f["enc3_b"][:FLAT], FLAT)
    cput("e3lvb", 0.5 * f["enc3_b"][FLAT:], FLAT)
    cput("projb8", _colmajor(f["proj_b"] * SX, 4))
    bpr = (f["proj_b"].astype(np.float64) @ f["router_w"].astype(np.float64)
           + f["router_b"].astype(np.float64)).astype(np.float32)
    cput("bpr", np.broadcast_to(bpr, (P, E)))
    cput("bgb", f["bg"].reshape(E, 8, P).transpose(2, 0, 1))
    cput("bub", f["bu"].reshape(E, 8, P).transpose(2, 0, 1) * SXW)
    cput("shgb", _colmajor(f["sh_g_b"], 16))
    cput("shub", _colmajor(f["sh_u_b"] * SXW, 16))
    cput("o1b", _colmajor(f["out1_b"], 8))
    cput("o2b", _colmajor(f["out2_b"], 8))
    cput("headb", f["head_b"], OUT)
    sh["cblob"] = cb
    bde17 = np.concatenate([f["bd"], f["sh_d_b"][None, :]], axis=0) * (SYD * SYD)
    sh["bde17"] = np.ascontiguousarray(bde17.astype(np.float32))
    sh["onesrow"] = np.full((1, BC), 1.0 / SYD, np.float32)

    maps = []
    for c in range(NCORES):
        s = slice(c * BC, (c + 1) * BC)
        m = dict(sh)
        m["xh1"] = np.ascontiguousarray(xh1[c])
        m["futt"] = np.ascontiguousarray(f["fut_ref"][s].T)
        m["obst"] = np.ascontiguousarray(f["cur_obs"][s].T)
        m["noiset"] = np.ascontiguousarray(f["vae_noise"][s].T)
        maps.append(m)
    return maps


last_exec_time_ns = None
last_results = None
last_res = None


def kernel(**inputs) -> np.ndarray:
    global last_exec_time_ns, last_results, last_res
    debug = bool(int(os.environ.get("KERNEL_DEBUG", "0")))
    key = ("dbg" if debug else "std")
    if key not in _CACHE:
        _CACHE[key] = _build(debug=debug)
    nc = _CACHE[key]
    maps = _prep(inputs)
    trace = bool(int(os.environ.get("KERNEL_TRACE", "0")))
    res = None
    for attempt in range(3):
        try:
            res = run_bass_kernel_spmd(nc, maps, list(range(NCORES)), trace=trace)
            break
        except Exception:
            if attempt == 2:
                raise
            import time as _time
            _time.sleep(20)
    last_exec_time_ns = res.exec_time_ns
    last_results = res.results
    last_res = res
    out = np.concatenate([res.results[c]["outp"].T for c in range(NCORES)], axis=0)
    return np.ascontiguousarray(out.astype(np.float32))


# revision 43
# speedup vs baseline: 1.0542x; 1.0542x over previous
"""Trainium2 Bass kernel for nn_EstVAEStudent (moe_routing).

Data-parallel over batch: 8 cores x 512 tokens, weights replicated.

v2: fp8e4 + DoubleRow matmuls (2x PE throughput, K=256/instruction) for the
conv encoder, hlin1, the 16 dense experts and the shared experts. The VAE
encoder, proj, router and output MLP stay float32r/fp32 (fp8 there flips
router selections). Expert down-projections run token-major so the routing
weights apply as per-partition scalars during PSUM eviction (no broadcast
DMAs), accumulating h2 = sum_e w_e y_e + w.bd + shared + sh_d_b in SBUF.
Router math is fp32 from an exact xcat copy with proj@router collapsed on
host; its top-k runs batched over all 512 tokens.

fp8 scaling: SX=8 (x), SW=32 (weights), so u-PSUMs sit at 256x and the
expert h stores as fp8 at 256*h (<240). Conv activations store at scale 1
(values <0.2), conv3 output + pooling in bf16.
"""

import os
import sys

sys.path.insert(0, "/opt/trn_rl_repo")

import numpy as np
import ml_dtypes

import concourse.bass as bass
import concourse.tile as tile
from concourse import bacc, mybir
from concourse.bass import ts
from concourse.bass_utils import run_bass_kernel_spmd
from concourse.masks import make_identity

F32 = mybir.dt.float32
F32R = mybir.dt.float32r
BF16 = mybir.dt.bfloat16
F8 = mybir.dt.float8e4
E4 = ml_dtypes.float8_e4m3
AF = mybir.ActivationFunctionType
ALU = mybir.AluOpType
AX = mybir.AxisListType
DR = mybir.MatmulPerfMode.DoubleRow

P = 128
NCORES = 8
B = 4096
BC = B // NCORES          # tokens per core = 512
TB = 128                  # conv token block
CUR_OBS = 256
HIST_C = 96
HIST_T = 25
FUT = 2560
HID = 1024
PROJ = 512
HLAT = 64
FLAT = 64
CONV1, CONV2, CONV3 = 256, 512, 1024
E = 16
NSH = 2
OUT = 23

SX = 8.0                  # x -> fp8 scale
SW = 32.0                 # fp8 weight scale
SXW = SX * SW             # u/g psum scale (256)
SYD = SXW * SW            # down-proj psum scale (8192)
SCIN = 16.0               # conv input scale
SPOOL = 256.0             # pooled -> fp8 scale

# bias-blob column layout: name -> (col, width)
_COFF = {}
_CB_COLS = 0
for _nm, _w in [("b1", 2), ("b2", 4), ("b3", 8), ("hl1b", 8), ("e1b", 8),
                ("e2b", 8), ("projb8", 4), ("bpr", E), ("bgb", E * 8),
                ("bub", E * 8), ("shgb", 16), ("shub", 16), ("o1b", 8),
                ("o2b", 8), ("hl2b", 1), ("e3mub", 1), ("e3lvb", 1),
                ("headb", 1)]:
    _COFF[_nm] = (_CB_COLS, _w)
    _CB_COLS += _w

_CACHE = {}


def _build(debug=False):
    nc = bacc.Bacc()
    dp = nc.declare_dram_parameter

    # ---- per-core inputs ----
    # host im2col conv1 input, block-contiguous: [block, ci, k, t'*TB+tok]
    xh1 = dp("xh1", (BC // TB, P, 3, HIST_T * TB), F8, isOutput=False)
    futt = dp("futt", (FUT, BC), F32, isOutput=False)
    obst = dp("obst", (CUR_OBS, BC), F32, isOutput=False)
    noiset = dp("noiset", (FLAT, BC), F32, isOutput=False)

    # ---- weights ----
    w18 = dp("w18", (P, 3, CONV1), F8, isOutput=False)    # [ci(96p), k, co]*SW
    w28 = dp("w28", (P, 3, 2, CONV2), F8, isOutput=False)
    w38 = dp("w38", (P, 3, 4, CONV3), F8, isOutput=False)
    hl1w8 = dp("hl1w8", (P, 8, HID), F8, isOutput=False)  # hlin1_w*SW (no /7)
    hl2w = dp("hl2w", (HID, HLAT), F32, isOutput=False)
    e1w = dp("e1w", (FUT, HID), F32, isOutput=False)
    e2w = dp("e2w", (HID, HID), F32, isOutput=False)
    e3w = dp("e3w", (HID, 2 * FLAT), F32, isOutput=False)
    projw = dp("projw", (CUR_OBS + HLAT + FLAT, PROJ), F32, isOutput=False)
    wpr = dp("wpr", (CUR_OBS + HLAT + FLAT, E), F32, isOutput=False)
    Wgu8 = dp("Wgu8", (E, P, 8, HID), F8, isOutput=False)  # [0:4]=Wg, [4:8]=Wu
    Wd8 = dp("Wd8", (E, P, 8, PROJ), F8, isOutput=False)
    shg8 = dp("shg8", (P, 4, NSH * HID), F8, isOutput=False)
    shu8 = dp("shu8", (P, 4, NSH * HID), F8, isOutput=False)
    shd8 = dp("shd8", (P, 16, PROJ), F8, isOutput=False)
    o1w = dp("o1w", (PROJ, HID), F32, isOutput=False)
    o2w = dp("o2w", (HID, HID), F32, isOutput=False)
    headw = dp("headw", (HID, OUT), F32, isOutput=False)

    # biases / consts packed into one blob (single DMA; see _COFF)
    cblob = dp("cblob", (P, _CB_COLS), F32, isOutput=False)
    bde17 = dp("bde17", (17, PROJ), F32, isOutput=False)  # [bd; sh_d_b]*SYD^2
    onesrow = dp("onesrow", (1, BC), F32, isOutput=False)  # 1/SYD

    outp = dp("outp", (OUT, BC), F32, isOutput=True)
    dbg = {}
    if debug:
        dbg["w"] = dp("dbg_w", (P, 4, E), F32, isOutput=True)
        dbg["h2"] = dp("dbg_h2", (P, 4, BC), F32, isOutput=True)
        dbg["x8"] = dp("dbg_x8", (P, 4, BC), F32, isOutput=True)
        dbg["macc"] = dp("dbg_macc", (P, 4, PROJ), F32, isOutput=True)

    with tile.TileContext(nc, pool_alloc_mode="queue") as tc:
        _emit(nc, tc, locals(), dbg)
    nc.compile()
    return nc


def _emit(nc, tc, t, dbg):
    import contextlib

    ctx = contextlib.ExitStack()
    with ctx:
        const = ctx.enter_context(tc.tile_pool(name="const", bufs=1))
        wpool = ctx.enter_context(tc.tile_pool(name="wpool", bufs=3))
        e1pool = ctx.enter_context(tc.tile_pool(name="e1pool", bufs=2))
        acts = ctx.enter_context(tc.tile_pool(name="acts", bufs=1))
        ps = ctx.enter_context(tc.tile_pool(name="ps", bufs=4, space="PSUM"))

        # ---------- constants (single blob DMA) ----------
        cb = const.tile([P, _CB_COLS], F32, tag="cblob")
        nc.sync.dma_start(cb[:], t["cblob"][:])

        def cbv(name, rows=P):
            a, w = _COFF[name]
            return cb[0:rows, a:a + w]

        b1s = cbv("b1"); b2s = cbv("b2"); b3s = cbv("b3")
        hl1bs = cbv("hl1b"); hl2bs = cbv("hl2b", HLAT)
        e1bs = cbv("e1b"); e2bs = cbv("e2b")
        e3mubs = cbv("e3mub", FLAT); e3lvbs = cbv("e3lvb", FLAT)
        projb8s = cbv("projb8"); bprs = cbv("bpr")
        bgbs = cbv("bgb").rearrange("p (e i) -> p e i", i=8)
        bubs = cbv("bub").rearrange("p (e i) -> p e i", i=8)
        shgbs = cbv("shgb"); shubs = cbv("shub")
        o1bs = cbv("o1b"); o2bs = cbv("o2b")
        headbs = cbv("headb", OUT)
        ident = const.tile([P, P], F32, tag="ident")
        make_identity(nc, ident)

        # bd-init rhs (f32r for 1cy/row); DMAs deferred past the conv weights
        bde17s = const.tile([17, PROJ], F32R, tag="bde17")
        # wTs17: transposed routing weights (rows 0-15) + const row 16 = 1/SYD
        wTs17 = acts.tile([17, BC], F32R, tag="wTs17")

        # persistent activations
        xcat = acts.tile([P, 3, BC], F32R, tag="xcat")
        xcat32 = acts.tile([P, 3, BC], F32, tag="xcat32")   # exact fp32 for router
        x8 = acts.tile([P, 4, BC], F8, tag="x8")

        # ---------- conv encoder (fp8 DR) + VAE e1 (f32r) interleaved ----------
        histp_cm = tc.tile_pool(name="histp", bufs=1)
        histp = histp_cm.__enter__()
        pooled = histp.tile([P, 8, BC], BF16, tag="pooled")
        vaep_cm = tc.tile_pool(name="vaep", bufs=1)
        vaep = vaep_cm.__enter__()
        futs = vaep.tile([P, FUT // P, BC], F32R, tag="futs")
        ee1 = vaep.tile([P, 8, BC], F32R, tag="ee1")

        e2pre = {}

        def emit_e1_mtile(m):
            nko = FUT // P
            wt = e1pool.tile([P, nko, P], F32R, tag="w", name="e1wt")
            nc.sync.dma_start(
                wt[:], t["e1w"].rearrange("(o p) m -> p o m", p=P)
                [:, :, ts(m, P)].bitcast(F32R))
            acc = ps.tile([P, 512], F32, tag="ps", name="acc")
            for ko in range(nko):
                nc.tensor.matmul(acc[:], lhsT=wt[:, ko, :],
                                 rhs=futs[:, ko, :],
                                 start=(ko == 0), stop=(ko == nko - 1))
            nc.scalar.activation(ee1[:, m, :], acc[:], AF.Silu, bias=e1bs[:, m:m + 1])

        with tc.tile_pool(name="convp", bufs=1) as convp, \
             tc.tile_pool(name="cps", bufs=2, space="PSUM") as cps:
            w1s = convp.tile([P, 3, CONV1], F8, tag="w1s")
            nc.sync.dma_start(w1s[:], t["w18"][:])
            w2s = convp.tile([P, 3, 2, CONV2], F8, tag="w2s")
            nc.sync.dma_start(w2s[:], t["w28"][:])
            w3s = convp.tile([P, 3, 4, CONV3], F8, tag="w3s")
            nc.sync.dma_start(w3s[:], t["w38"][:])

            for b in range(BC // TB):
                # host-im2col conv1 input: [ci(96), k, c = t'*TB + tok]
                x1 = convp.tile([P, 3, 25 * TB], F8, tag="cX")
                nc.sync.dma_start(x1[:], t["xh1"][b])
                if b == 0:
                    for fc in range(4):
                        nc.sync.dma_start(
                            futs[:, ts(fc, 5), :],
                            t["futt"].rearrange("(o p) n -> p o n", p=P)[:, ts(fc, 5), :].bitcast(F32R))
                    nc.sync.dma_start(
                        xcat[:, 0:2, :],
                        t["obst"].rearrange("(o p) n -> p o n", p=P).bitcast(F32R))
                    nc.sync.dma_start(
                        xcat32[:, 0:2, :],
                        t["obst"].rearrange("(o p) n -> p o n", p=P))
                    nc.sync.dma_start(bde17s[:], t["bde17"][:].bitcast(F32R))
                    nc.sync.dma_start(wTs17[16:17, :], t["onesrow"][:].bitcast(F32R))

                # conv1 out, t-major with pad rows 0 and 26: [co-tile, (t'+1)*TB + tok]
                s1t = convp.tile([P, 2, 27 * TB], F8, tag="cA")
                nc.vector.memset(s1t[:, :, 0:TB], 0.0)
                nc.vector.memset(s1t[:, :, 26 * TB:27 * TB], 0.0)
                C1 = 25 * TB
                for co in range(2):
                    for cbase in range(0, C1, 1024):
                        w = min(1024, C1 - cbase)
                        acc = cps.tile([P, 1024], F32, tag="cps", name="acc")
                        for c0 in range(0, w, 512):
                            cw = min(512, w - c0)
                            sub = acc[:, c0:c0 + cw]
                            nc.tensor.matmul(sub, lhsT=w1s[0:96, 0:2, ts(co, P)],
                                             rhs=x1[0:96, 0:2, cbase + c0:cbase + c0 + cw],
                                             start=True, stop=False, perf_mode=DR,
                                             skip_group_check=True)
                            nc.tensor.matmul(sub, lhsT=w1s[0:96, 2, ts(co, P)],
                                             rhs=x1[0:96, 2, cbase + c0:cbase + c0 + cw],
                                             start=False, stop=True, skip_group_check=True)
                        nc.scalar.activation(s1t[:, co, TB + cbase:TB + cbase + w],
                                             acc[:, 0:w], AF.Silu,
                                             bias=b1s[:, co:co + 1], scale=1.0 / (SCIN * SW))

                # on-chip im2col for conv2: s1im[ci, k, t''*TB+tok] = s1t[ci, (2t''+k)*TB+tok]
                # one 4D-AP DMA per ci (k, t'', tok), issued from the idle gpsimd DGE
                s1im = convp.tile([P, 2, 3, 13 * TB], F8, tag="cX")
                for ci in range(2):
                    s1v = s1t[:, ci, :].rearrange("p (t n) -> p t n", n=TB)
                    for k in range(3):
                        nc.sync.dma_start(s1im[:, ci, k, :], s1v[:, slice(k, k + 25, 2), :])

                # conv2: stride 2, ci 256 = DR pair over partition tiles
                s2t = convp.tile([P, 4, 15 * TB], F8, tag="cB")
                nc.vector.memset(s2t[:, :, 0:TB], 0.0)
                nc.vector.memset(s2t[:, :, 14 * TB:15 * TB], 0.0)
                C2 = 13 * TB
                for co in range(4):
                    for cbase in range(0, C2, 1024):
                        w = min(1024, C2 - cbase)
                        acc = cps.tile([P, 1024], F32, tag="cps", name="acc")
                        for c0 in range(0, w, 512):
                            cw = min(512, w - c0)
                            for k in range(3):
                                nc.tensor.matmul(
                                    acc[:, c0:c0 + cw], lhsT=w2s[:, k, :, ts(co, P)],
                                    rhs=s1im[:, :, k, cbase + c0:cbase + c0 + cw],
                                    start=(k == 0), stop=(k == 2), perf_mode=DR,
                                    skip_group_check=True)
                        nc.scalar.activation(s2t[:, co, TB + cbase:TB + cbase + w],
                                             acc[:, 0:w], AF.Silu,
                                             bias=b2s[:, co:co + 1], scale=1.0 / SW)

                # im2col for conv3
                s2im = convp.tile([P, 4, 3, 7 * TB], F8, tag="cA")
                for ci in range(4):
                    s2v = s2t[:, ci, :].rearrange("p (t n) -> p t n", n=TB)
                    for k in range(3):
                        nc.sync.dma_start(s2im[:, ci, k, :], s2v[:, slice(k, k + 13, 2), :])

                # conv3: stride 2, ci 512 = 2 DR pairs
                s3 = convp.tile([P, 8, 7 * TB], BF16, tag="cB")
                C3 = 7 * TB
                for co in range(8):
                    acc = cps.tile([P, 1024], F32, tag="cps", name="acc")
                    for c0 in range(0, C3, 512):
                        cw = min(512, C3 - c0)
                        first = True
                        for cp in range(2):
                            for k in range(3):
                                nc.tensor.matmul(
                                    acc[:, c0:c0 + cw],
                                    lhsT=w3s[:, k, slice(2 * cp, 2 * cp + 2), ts(co, P)],
                                    rhs=s2im[:, slice(2 * cp, 2 * cp + 2), k, c0:c0 + cw],
                                    start=first, stop=(cp == 1 and k == 2), perf_mode=DR,
                                    skip_group_check=True)
                                first = False
                    nc.scalar.activation(s3[:, co, :], acc[:, 0:C3], AF.Silu,
                                         bias=b3s[:, co:co + 1], scale=1.0 / SW)
                # pool (sum over t; /7 folded into pooled8 cast)
                s3v = s3.rearrange("p c (t n) -> p c t n", n=TB)
                pb = pooled[:, :, ts(b, TB)]
                with nc.allow_low_precision(reason="bf16 pooling"):
                    nc.vector.tensor_add(pb, s3v[:, :, 0, :], s3v[:, :, 1, :])
                    for tt_ in range(2, 7):
                        nc.vector.tensor_add(pb, pb, s3v[:, :, tt_, :])
                emit_e1_mtile(2 * b)
                emit_e1_mtile(2 * b + 1)
                if b == 2:
                    for m_ in range(3):
                        wt = wpool.tile([P, 8, P], F32R, tag="wsm", name="wt")
                        nc.sync.dma_start(
                            wt[:], t["e2w"].rearrange("(o p) m -> p o m", p=P)
                            [:, :, ts(m_, P)].bitcast(F32R))
                        e2pre[m_] = wt

        # post-conv persistent pool (reuses the conv pools' SBUF space);
        # right-side stack so it can outlive the left-side histp/vaep pools
        acts2 = ctx.enter_context(tc.tile_pool(name="acts2", bufs=1, side="right"))
        hl1w8s = acts2.tile([P, 8, HID], F8, tag="hl1w8")
        nc.sync.dma_start(hl1w8s[:], t["hl1w8"][:])
        shg8s = acts2.tile([P, 4, NSH * HID], F8, tag="shg8")
        nc.sync.dma_start(shg8s[:], t["shg8"][:])
        shu8s = acts2.tile([P, 4, NSH * HID], F8, tag="shu8")
        nc.sync.dma_start(shu8s[:], t["shu8"][:])
        shd8s = acts2.tile([P, 16, PROJ], F8, tag="shd8")
        nc.sync.dma_start(shd8s[:], t["shd8"][:])

        # ---------- VAE e2/e3 (f32r) + hist MLP ----------
        with tc.tile_pool(name="vtmp", bufs=1) as vtmp:
            ee2 = vtmp.tile([P, 8, BC], F32R, tag="ee2")
            for m in range(8):
                if m in e2pre:
                    wt = e2pre[m]
                else:
                    wt = wpool.tile([P, 8, P], F32R, tag="wsm", name="wt")
                    nc.sync.dma_start(
                        wt[:], t["e2w"].rearrange("(o p) m -> p o m", p=P)[:, :, ts(m, P)].bitcast(F32R))
                acc = ps.tile([P, 512], F32, tag="ps", name="acc")
                for ko in range(8):
                    nc.tensor.matmul(acc[:], lhsT=wt[:, ko, :], rhs=ee1[:, ko, :],
                                     start=(ko == 0), stop=(ko == 7))
                nc.scalar.activation(ee2[:, m, :], acc[:], AF.Silu, bias=e2bs[:, m:m + 1])
            # hist MLP: pooled -> fp8 cast, hl1 in fp8-DR, hl2 f32r
            pooled8 = vtmp.tile([P, 8, BC], F8, tag="pooled8")
            nc.vector.tensor_scalar_mul(pooled8[:], pooled[:], SPOOL / 7.0)
            h1 = vtmp.tile([P, 8, BC], F32R, tag="h1")
            for m in range(8):
                acc = ps.tile([P, 512], F32, tag="ps", name="acc")
                for j in range(4):
                    nc.tensor.matmul(acc[:], lhsT=hl1w8s[:, 2 * j:2 * j + 2, ts(m, P)],
                                     rhs=pooled8[:, 2 * j:2 * j + 2, :],
                                     start=(j == 0), stop=(j == 3), perf_mode=DR)
                nc.scalar.activation(h1[:, m, :], acc[:], AF.Silu, bias=hl1bs[:, m:m + 1],
                                     scale=1.0 / (SPOOL * SW))
            wt = wpool.tile([P, 8, HLAT], F32R, tag="wsm", name="wt")
            nc.sync.dma_start(wt[:], t["hl2w"].rearrange("(o p) m -> p o m", p=P).bitcast(F32R))
            acc = ps.tile([P, 512], F32, tag="ps", name="acc")[:HLAT, :]
            for ko in range(8):
                nc.tensor.matmul(acc, lhsT=wt[:, ko, :], rhs=h1[:, ko, :],
                                 start=(ko == 0), stop=(ko == 7))
            nc.scalar.activation(xcat32[0:HLAT, 2, :], acc, AF.Identity, bias=hl2bs[:])
            nc.vector.tensor_copy(xcat[0:HLAT, 2, :], xcat32[0:HLAT, 2, :])
            # enc3 -> mu / logvar
            wt = wpool.tile([P, 8, 2 * FLAT], F32R, tag="wsm", name="wt")
            nc.sync.dma_start(wt[:], t["e3w"].rearrange("(o p) m -> p o m", p=P).bitcast(F32R))
            accmu = ps.tile([P, 512], F32, tag="ps", name="acc")[:FLAT, :]
            for ko in range(8):
                nc.tensor.matmul(accmu, lhsT=wt[:, ko, 0:FLAT], rhs=ee2[:, ko, :],
                                 start=(ko == 0), stop=(ko == 7))
            mus = vtmp.tile([FLAT, BC], F32, tag="mu")
            nc.scalar.activation(mus[:], accmu, AF.Identity, bias=e3mubs[:])
            acclv = ps.tile([P, 512], F32, tag="ps", name="acc")[:FLAT, :]
            for ko in range(8):
                nc.tensor.matmul(acclv, lhsT=wt[:, ko, FLAT:2 * FLAT], rhs=ee2[:, ko, :],
                                 start=(ko == 0), stop=(ko == 7))
            zexp = vtmp.tile([FLAT, BC], F32, tag="zexp")
            nc.scalar.activation(zexp[:], acclv, AF.Exp, bias=e3lvbs[:], scale=0.5)
            noises = vtmp.tile([FLAT, BC], F32, tag="noise")
            nc.sync.dma_start(noises[:], t["noiset"][:])
            nc.vector.tensor_mul(zexp[:], zexp[:], noises[:])
            nc.vector.tensor_add(zexp[:], zexp[:], mus[:])
            nc.sync.dma_start(xcat[HLAT:P, 2, :], zexp[:].bitcast(F32R))
            nc.sync.dma_start(xcat32[HLAT:P, 2, :], zexp[:])
        vaep_cm.__exit__(None, None, None)
        histp_cm.__exit__(None, None, None)

        # ---------- projection -> x8 ----------
        for m in range(4):
            wt = wpool.tile([P, 3, P], F32R, tag="wsm", name="wt")
            nc.sync.dma_start(
                wt[:], t["projw"].rearrange("(o p) m -> p o m", p=P)[:, :, ts(m, P)].bitcast(F32R))
            acc = ps.tile([P, 512], F32, tag="ps")
            for ko in range(3):
                nc.tensor.matmul(acc[:], lhsT=wt[:, ko, :], rhs=xcat[:, ko, :],
                                 start=(ko == 0), stop=(ko == 2))
            nc.scalar.activation(x8[:, m, :], acc[:], AF.Identity,
                                 bias=projb8s[:, m:m + 1], scale=SX)

        if dbg:
            x8dbg = acts2.tile([P, 4, BC], F32, tag="dbgx8", name="x8dbg")
            nc.scalar.activation(x8dbg[:], x8[:], AF.Identity, scale=1.0 / SX)
            nc.sync.dma_start(dbg["x8"][:], x8dbg[:])

        # ---------- router (fp32, batched over the 4 token tiles) ----------
        rws = const.tile([P, 3, E], F32, tag="rws")
        nc.sync.dma_start(rws[:], t["wpr"].rearrange("(o p) m -> p o m", p=P))
        wfin = acts.tile([P, 4, E], F32, tag="wfin")
        with tc.tile_pool(name="routp", bufs=1) as routp, \
             tc.tile_pool(name="pss", bufs=2, space="PSUM") as pss:
            sc = routp.tile([P, 4, E], F32, tag="sc")
            for tt in range(4):
                acc = pss.tile([P, E], F32, tag="rps")
                for ko in range(3):
                    nc.tensor.matmul(acc[:], lhsT=xcat32[:, ko, ts(tt, P)], rhs=rws[:, ko, :],
                                     start=(ko == 0), stop=(ko == 2))
                nc.vector.tensor_add(sc[:, tt, :], acc[:], bprs[:])
            nc.scalar.activation(sc[:], sc[:], AF.Sigmoid)
            scg = sc.rearrange("p f (g i) -> p f g i", g=4)
            # group score: sum of top2 of 4 = max over pairwise sums
            pa = routp.tile([P, 4, 4, 3], F32, tag="pa")
            nc.vector.tensor_add(pa[:], scg[:, :, :, 0:3], scg[:, :, :, 1:4])
            pb = routp.tile([P, 4, 4, 2], F32, tag="pb")
            nc.vector.tensor_add(pb[:], scg[:, :, :, 0:2], scg[:, :, :, 2:4])
            pc = routp.tile([P, 4, 4, 1], F32, tag="pc")
            nc.vector.tensor_add(pc[:], scg[:, :, :, 0:1], scg[:, :, :, 3:4])
            gsc = routp.tile([P, 4, 4], F32, tag="gsc")
            nc.vector.reduce_max(gsc[:], pa[:], axis=AX.X)
            pbm = routp.tile([P, 4, 4], F32, tag="pbm")
            nc.vector.reduce_max(pbm[:], pb[:], axis=AX.X)
            nc.vector.tensor_max(gsc[:], gsc[:], pbm[:])
            nc.vector.tensor_max(gsc[:], gsc[:], pc[:, :, :, 0])
            # top-2 groups by count-greater
            cg = routp.tile([P, 4, 4], F32, tag="cg")
            nc.vector.memset(cg[:], 0.0)
            tmp = routp.tile([P, 4, 4], F32, tag="tmpr")
            for d in range(1, 4):
                nc.vector.tensor_tensor(tmp[:, :, :4 - d], gsc[:, :, d:], gsc[:, :, :4 - d], ALU.is_gt)
                nc.vector.tensor_add(cg[:, :, :4 - d], cg[:, :, :4 - d], tmp[:, :, :4 - d])
                nc.vector.tensor_tensor(tmp[:, :, :4 - d], gsc[:, :, :4 - d], gsc[:, :, d:], ALU.is_gt)
                nc.vector.tensor_add(cg[:, :, d:], cg[:, :, d:], tmp[:, :, :4 - d])
            gmask = routp.tile([P, 4, 4], F32, tag="gmask")
            nc.vector.tensor_scalar(gmask[:], cg[:], 1.5, None, ALU.is_le)
            msk = routp.tile([P, 4, 4, 4], F32, tag="msk")
            nc.vector.tensor_tensor(msk[:], scg[:], gmask[:, :, :, None].to_broadcast((P, 4, 4, 4)), ALU.mult)
            mskf = msk.rearrange("p f g i -> p f (g i)")
            # top-4 of 16 by count-greater
            cg16 = routp.tile([P, 4, E], F32, tag="cg16")
            nc.vector.memset(cg16[:], 0.0)
            t16 = routp.tile([P, 4, E], F32, tag="t16")
            for d in range(1, 16):
                nc.vector.tensor_tensor(t16[:, :, :E - d], mskf[:, :, d:], mskf[:, :, :E - d], ALU.is_gt)
                nc.vector.tensor_add(cg16[:, :, :E - d], cg16[:, :, :E - d], t16[:, :, :E - d])
                nc.vector.tensor_tensor(t16[:, :, :E - d], mskf[:, :, :E - d], mskf[:, :, d:], ALU.is_gt)
                nc.vector.tensor_add(cg16[:, :, d:], cg16[:, :, d:], t16[:, :, :E - d])
            sel = routp.tile([P, 4, E], F32, tag="sel")
            nc.vector.tensor_scalar(sel[:], cg16[:], 3.5, None, ALU.is_le)
            wsel = routp.tile([P, 4, E], F32, tag="wsel")
            nc.vector.tensor_tensor(wsel[:], mskf[:], sel[:], ALU.mult)
            den = routp.tile([P, 4], F32, tag="den")
            nc.vector.reduce_sum(den[:], wsel[:], axis=AX.X)
            nc.vector.tensor_scalar_add(den[:], den[:], 1e-20)
            nc.vector.tensor_scalar_mul(den[:], den[:], SYD)
            rec = routp.tile([P, 4], F32, tag="rec")
            nc.vector.reciprocal(rec[:], den[:])
            # wfin = wsel / (SYD * den)  (per-token scale broadcast over E)
            nc.vector.tensor_tensor(wfin[:], wsel[:], rec[:, :, None].to_broadcast((P, 4, E)), ALU.mult)
            # transpose routing weights into wTs17 rows 0-15
            for tt in range(4):
                acct = pss.tile([E, P], F32, tag="tps")
                nc.tensor.transpose(acct[:], wfin[:, tt, :], ident[:])
                with nc.allow_low_precision(reason="f32r copy of router weights"):
                    nc.vector.tensor_copy(wTs17[0:16, ts(tt, P)], acct[:].bitcast(F32R))

        if dbg:
            nc.sync.dma_start(dbg["w"][:], wfin[:])

        # ---------- shared experts (fp8 DR) + bd-init, token-major down ----------
        # psum pool for token-major down-projections (router pss closed by now)
        psy = ctx.enter_context(tc.tile_pool(name="psy", bufs=2, space="PSUM"))
        macc = acts2.tile([P, 4, PROJ], F32, tag="macc")  # [tok, tt, proj]
        with tc.tile_pool(name="hshp", bufs=1) as hshp, \
             tc.tile_pool(name="hgp", bufs=3) as hgp:
            hsh8 = hshp.tile([P, 16, BC], F8, tag="hsh8")
            for i in range(16):
                gacc = ps.tile([P, 512], F32, tag="ps")
                for j in range(2):
                    nc.tensor.matmul(gacc[:], lhsT=shg8s[:, 2 * j:2 * j + 2, ts(i, P)],
                                     rhs=x8[:, 2 * j:2 * j + 2, :],
                                     start=(j == 0), stop=(j == 1), perf_mode=DR)
                hgt = hgp.tile([P, BC], F32, tag="hgt")
                nc.scalar.activation(hgt[:], gacc[:], AF.Silu, bias=shgbs[:, i:i + 1],
                                     scale=1.0 / SXW)
                uacc = ps.tile([P, 512], F32, tag="ps")
                for j in range(2):
                    nc.tensor.matmul(uacc[:], lhsT=shu8s[:, 2 * j:2 * j + 2, ts(i, P)],
                                     rhs=x8[:, 2 * j:2 * j + 2, :],
                                     start=(j == 0), stop=(j == 1), perf_mode=DR)
                nc.vector.scalar_tensor_tensor(hsh8[:, i, :], uacc[:], shubs[:, i:i + 1],
                                               hgt[:], ALU.add, ALU.mult)
            # token-major: psum_tt = bd-init (K=17) + shared down; evict -> macc
            for tt in range(4):
                yacc = psy.tile([P, 512], F32, tag="psy")
                nc.tensor.matmul(yacc[:], lhsT=wTs17[:, ts(tt, P)], rhs=bde17s[:],
                                 start=True, stop=False, skip_group_check=True)
                for j in range(8):
                    nc.tensor.matmul(yacc[:], lhsT=hsh8[:, 2 * j:2 * j + 2, ts(tt, P)],
                                     rhs=shd8s[:, 2 * j:2 * j + 2, :],
                                     start=False, stop=(j == 7), perf_mode=DR,
                                     skip_group_check=True)
                nc.scalar.activation(macc[:, tt, :], yacc[:], AF.Identity, scale=1.0 / SYD)

        # ---------- 16 experts (fp8 DR), token-major down + fused routing ----------
        with tc.tile_pool(name="moep", bufs=3) as moep, \
             tc.tile_pool(name="h8p", bufs=2) as h8p, \
             tc.tile_pool(name="hgp2", bufs=3) as hgp2:
            for e in range(E):
                wgu = moep.tile([P, 8, HID], F8, tag="wgu")
                nc.sync.dma_start(wgu[:], t["Wgu8"][e])
                wg = wgu[:, 0:4, :]
                wu = wgu[:, 4:8, :]
                wd = moep.tile([P, 8, PROJ], F8, tag="wd")
                nc.sync.dma_start(wd[:], t["Wd8"][e])
                h8 = h8p.tile([P, 8, BC], F8, tag="h8")
                for i in range(8):
                    gacc = ps.tile([P, 512], F32, tag="ps")
                    for j in range(2):
                        nc.tensor.matmul(gacc[:], lhsT=wg[:, 2 * j:2 * j + 2, ts(i, P)],
                                         rhs=x8[:, 2 * j:2 * j + 2, :],
                                         start=(j == 0), stop=(j == 1), perf_mode=DR)
                    hgt = hgp2.tile([P, BC], F32, tag="hgt")
                    nc.scalar.activation(hgt[:], gacc[:], AF.Silu, bias=bgbs[:, e, i:i + 1],
                                         scale=1.0 / SXW)
                    uacc = ps.tile([P, 512], F32, tag="ps")
                    for j in range(2):
                        nc.tensor.matmul(uacc[:], lhsT=wu[:, 2 * j:2 * j + 2, ts(i, P)],
                                         rhs=x8[:, 2 * j:2 * j + 2, :],
                                         start=(j == 0), stop=(j == 1), perf_mode=DR)
                    nc.vector.scalar_tensor_tensor(h8[:, i, :], uacc[:], bubs[:, e, i:i + 1],
                                                   hgt[:], ALU.add, ALU.mult)
                for tt in range(4):
                    yacc = psy.tile([P, 512], F32, tag="psy")
                    for j in range(4):
                        nc.tensor.matmul(yacc[:], lhsT=h8[:, 2 * j:2 * j + 2, ts(tt, P)],
                                         rhs=wd[:, 2 * j:2 * j + 2, :],
                                         start=(j == 0), stop=(j == 3), perf_mode=DR)
                    # macc += y * w_e(token)   (w pre-divided by SYD)
                    nc.vector.scalar_tensor_tensor(macc[:, tt, :], yacc[:],
                                                   wfin[:, tt, e:e + 1], macc[:, tt, :],
                                                   ALU.mult, ALU.add)

        if dbg:
            nc.sync.dma_start(dbg["macc"][:], macc[:])

        # ---------- transpose h2 to feature-major ----------
        h2 = acts2.tile([P, 4, BC], F32R, tag="h2")
        with tc.tile_pool(name="pst", bufs=2, space="PSUM") as pstp:
            for tt in range(4):
                for fc in range(4):
                    pst = pstp.tile([P, P], F32, tag="pst")
                    nc.tensor.transpose(pst[:], macc[:, tt, ts(fc, P)], ident[:])
                    nc.scalar.activation(h2[:, fc, ts(tt, P)], pst[:], AF.Identity)

        if dbg:
            nc.sync.dma_start(dbg["h2"][:], h2[:].bitcast(F32))

        # ---------- output MLP + head (f32r) ----------
        with tc.tile_pool(name="outp_", bufs=2) as outp_:
            o1 = outp_.tile([P, 8, BC], F32R, tag="o")
            for m in range(8):
                wt = wpool.tile([P, 4, P], F32R, tag="wsm", name="wt")
                nc.sync.dma_start(
                    wt[:], t["o1w"].rearrange("(o p) m -> p o m", p=P)[:, :, ts(m, P)].bitcast(F32R))
                acc = ps.tile([P, 512], F32, tag="ps")
                for ko in range(4):
                    nc.tensor.matmul(acc[:], lhsT=wt[:, ko, :], rhs=h2[:, ko, :],
                                     start=(ko == 0), stop=(ko == 3))
                nc.scalar.activation(o1[:, m, :], acc[:], AF.Silu, bias=o1bs[:, m:m + 1])
            o2 = outp_.tile([P, 8, BC], F32R, tag="o")
            for m in range(8):
                wt = wpool.tile([P, 8, P], F32R, tag="wsm", name="wt")
                nc.sync.dma_start(
                    wt[:], t["o2w"].rearrange("(o p) m -> p o m", p=P)[:, :, ts(m, P)].bitcast(F32R))
                acc = ps.tile([P, 512], F32, tag="ps")
                for ko in range(8):
                    nc.tensor.matmul(acc[:], lhsT=wt[:, ko, :], rhs=o1[:, ko, :],
                                     start=(ko == 0), stop=(ko == 7))
                nc.scalar.activation(o2[:, m, :], acc[:], AF.Identity, bias=o2bs[:, m:m + 1])
            wt = wpool.tile([P, 8, OUT], F32R, tag="wsm", name="wt")
            nc.sync.dma_start(wt[:], t["headw"].rearrange("(o p) m -> p o m", p=P).bitcast(F32R))
            acc = ps.tile([P, 512], F32, tag="ps", name="acc")[:OUT, :]
            for ko in range(8):
                nc.tensor.matmul(acc, lhsT=wt[:, ko, :], rhs=o2[:, ko, :],
                                 start=(ko == 0), stop=(ko == 7))
            outs = outp_.tile([OUT, BC], F32, tag="outs")
            nc.scalar.activation(outs[:], acc, AF.Identity, bias=headbs[:])
            nc.sync.dma_start(t["outp"][:], outs[:])


def _colmajor(v, ntiles):
    return np.ascontiguousarray(v.reshape(ntiles, P).T)


def _q8(a, s):
    return np.asarray(np.asarray(a, np.float32) * s, E4)


def _prep(inputs):
    f = {k: np.ascontiguousarray(np.asarray(v, dtype=np.float32)) for k, v in inputs.items()}
    sh = {}
    # hist: [B,25,96] -> padded [128(ci), 27(t), B], then host im2col
    # xh1[ci, k, t', tok] = xpad[ci, t' + k, tok], quantized fp8 * SCIN
    xh = np.zeros((P, 27, B), np.float32)
    xh[:HIST_C, 1:26, :] = f["cur_hist_seq"].transpose(2, 1, 0)
    xim = np.stack([xh[:, k:k + HIST_T, :] for k in range(3)], axis=1)  # [P,3,25,B]
    # -> [core, block, P, 3, 25*TB] block-contiguous
    xim = xim.reshape(P, 3, HIST_T, NCORES, BC // TB, TB).transpose(3, 4, 0, 1, 2, 5)
    xh1 = _q8(np.ascontiguousarray(xim.reshape(NCORES, BC // TB, P, 3, HIST_T * TB)), SCIN)
    # conv weights: [co,ci,k] -> [ci(part), k, co] * SW
    w1 = np.zeros((P, 3, CONV1), np.float32)
    w1[:HIST_C] = f["conv1_w"].transpose(1, 2, 0)
    sh["w18"] = _q8(w1, SW)
    sh["w28"] = _q8(np.ascontiguousarray(
        f["conv2_w"].transpose(1, 2, 0).reshape(2, P, 3, CONV2).transpose(1, 2, 0, 3)), SW)
    sh["w38"] = _q8(np.ascontiguousarray(
        f["conv3_w"].transpose(1, 2, 0).reshape(4, P, 3, CONV3).transpose(1, 2, 0, 3)), SW)
    sh["hl1w8"] = _q8(np.ascontiguousarray(
        f["hlin1_w"].reshape(8, P, HID).transpose(1, 0, 2)), SW)
    sh["hl2w"] = f["hlin2_w"]
    sh["e1w"] = f["enc1_w"]; sh["e2w"] = f["enc2_w"]; sh["e3w"] = f["enc3_w"]
    sh["projw"] = f["proj_w"]
    sh["wpr"] = (f["proj_w"].astype(np.float64) @ f["router_w"].astype(np.float64)).astype(np.float32)
    wgu = np.concatenate([f["Wg"].reshape(E, 4, P, HID), f["Wu"].reshape(E, 4, P, HID)],
                         axis=1)
    sh["Wgu8"] = _q8(wgu.transpose(0, 2, 1, 3), SW)
    sh["Wd8"] = _q8(f["Wd"].reshape(E, 8, P, PROJ).transpose(0, 2, 1, 3), SW)
    sh["shg8"] = _q8(f["sh_g_w"].reshape(4, P, NSH * HID).transpose(1, 0, 2), SW)
    sh["shu8"] = _q8(f["sh_u_w"].reshape(4, P, NSH * HID).transpose(1, 0, 2), SW)
    sh["shd8"] = _q8(f["sh_d_w"].reshape(16, P, PROJ).transpose(1, 0, 2), SW)
    sh["o1w"] = f["out1_w"]; sh["o2w"] = f["out2_w"]; sh["headw"] = f["head_w"]
    cb = np.zeros((P, _CB_COLS), np.float32)

    def cput(name, arr, rows=P):
        a, w = _COFF[name]
        cb[0:rows, a:a + w] = arr.reshape(rows, w)

    cput("b1", _colmajor(f["conv1_b"], 2))
    cput("b2", _colmajor(f["conv2_b"], 4))
    cput("b3", _colmajor(f["conv3_b"], 8))
    cput("hl1b", _colmajor(f["hlin1_b"], 8))
    cput("hl2b", f["hlin2_b"], HLAT)
    cput("e1b", _colmajor(f["enc1_b"], 8))
    cput("e2b", _colmajor(f["enc2_b"], 8))
    cput("e3mub", f["enc3_b"][:FLAT], FLAT)
    cput("e3lvb", 0.5 * f["enc3_b"][FLAT:], FLAT)
    cput("projb8", _colmajor(f["proj_b"] * SX, 4))
    bpr = (f["proj_b"].astype(np.float64) @ f["router_w"].astype(np.float64)
           + f["router_b"].astype(np.float64)).astype(np.float32)
    cput("bpr", np.broadcast_to(bpr, (P, E)))
    cput("bgb", f["bg"].reshape(E, 8, P).transpose(2, 0, 1))
    cput("bub", f["bu"].reshape(E, 8, P).transpose(2, 0, 1) * SXW)
    cput("shgb", _colmajor(f["sh_g_b"], 16))
    cput("shub", _colmajor(f["sh_u_b"] * SXW, 16))
    cput("o1b", _colmajor(f["out1_b"], 8))
    cput("o2b", _colmajor(f["out2_b"], 8))
    cput("headb", f["head_b"], OUT)
    sh["cblob"] = cb
    bde17 = np.concatenate([f["bd"], f["sh_d_b"][None, :]], axis=0) * (SYD * SYD)
    sh["bde17"] = np.ascontiguousarray(bde17.astype(np.float32))
    sh["onesrow"] = np.full((1, BC), 1.0 / SYD, np.float32)

    maps = []
    for c in range(NCORES):
        s = slice(c * BC, (c + 1) * BC)
        m = dict(sh)
        m["xh1"] = np.ascontiguousarray(xh1[c])
        m["futt"] = np.ascontiguousarray(f["fut_ref"][s].T)
        m["obst"] = np.ascontiguousarray(f["cur_obs"][s].T)
        m["noiset"] = np.ascontiguousarray(f["vae_noise"][s].T)
        maps.append(m)
    return maps


last_exec_time_ns = None
last_results = None
last_res = None


def kernel(**inputs) -> np.ndarray:
    global last_exec_time_ns, last_results, last_res
    debug = bool(int(os.environ.get("KERNEL_DEBUG", "0")))
    key = ("dbg" if debug else "std")
    if key not in _CACHE:
        _CACHE[key] = _build(debug=debug)
    nc = _CACHE[key]
    maps = _prep(inputs)
    trace = bool(int(os.environ.get("KERNEL_TRACE", "0")))
    res = None
    for attempt in range(3):
        try:
            res = run_bass_kernel_spmd(nc, maps, list(range(NCORES)), trace=trace)
            break
        except Exception:
            if attempt == 2:
                raise
            import time as _time
            _time.sleep(20)
    last_exec_time_ns = res.exec_time_ns
    last_results = res.results
    last_res = res
    out = np.concatenate([res.results[c]["outp"].T for c in range(NCORES)], axis=0)
    return np.ascontiguousarray(out.astype(np.float32))
